# revision 1
# baseline (speedup 1.0000x reference)
"""Causal single-head attention on 8 trn2 NeuronCores.

Sharding: core c handles batch c//2 and half the query rows of that batch
(4 blocks of 256 rows, picked so causal work balances). The device program is
identical on every core; which rows a core owns is data (host-side
gather/scatter + per-core causal masks).

Algorithm (v2) — projections folded away:
  scores = x (Wq^T Wk) x^T and out = P x Wv, so the device never forms
  Q, K, or V:
    host:  A = Wq^T @ Wk  (f32)
    dev:   T^T = A^T x^T  over own queries            (xA)
           per query-block: S^T[j,i] = x^T.T_tiles @ T^T   (PSUM)
             + causal mask add, probsT = exp(S^T/32)  (no max needed:
               scaled scores are O(+-2))
           U^T[d,i] += x_nat_tile.T @ probsT   accumulated over j-tiles
           l[1,i]   += ones.T @ probsT         (softmax denominator)
           U^T /= l (broadcast) -> sbuf, then out = (U^T).T @ Wv^T tiles.
"""

import sys

try:
    import concourse  # noqa: F401
except ImportError:
    sys.path.insert(0, "/opt/trn_rl_repo")

from contextlib import ExitStack

import ml_dtypes
import numpy as np

import concourse.bass as bass
from concourse import bacc
import concourse.mybir as mybir
import concourse.tile as tile
from concourse.bass_utils import run_bass_kernel_spmd

B, N, D = 4, 2048, 1024
NQ = 1024            # query rows owned per core
NCORES = 8
TRIPS = (4, 8, 12, 16)          # j-tile trip count per slot (uniform program)
SLOTS = ((0, 2, 4, 6), (1, 3, 5, 7))  # 256-row block owned by slot s, per h
SCALE = 1.0 / 32.0   # 1/sqrt(D)
IB = 256             # query block width
MDT = mybir.dt.bfloat16
NPDT = ml_dtypes.bfloat16

TRACE = False
LAST_RESULT = None
LAST_IN_MAPS = None
_CACHED_NC = None


def _qrows(h):
    return np.concatenate([np.arange(256 * p, 256 * p + 256) for p in SLOTS[h]])


def _build_masks(h):
    """[4 slots, 4, 128, 256] f32: additive causal masks for the last 4 j-tiles
    of each slot (covers the diagonal tiles and the padded tiles)."""
    masks = np.zeros((4, 4, 128, IB), np.float32)
    jp = np.arange(128)[:, None]
    iv = np.arange(IB)[None, :]
    for s in range(4):
        r0 = 256 * SLOTS[h][s]
        for k in range(4):
            jt = TRIPS[s] - 4 + k
            masks[s, k] = np.where(jt * 128 + jp <= r0 + iv, 0.0, -1e30)
    return masks


def _build_body(nc, tc, ctx, dram, rep):
    P = 128
    n_d = D // P          # 8
    n_j = N // P          # 16
    xt_w = N + NQ
    xt_d, xn_d, a_d, wvt_d, mask_d, out_d = dram
    r = rep

    pool_xt = ctx.enter_context(tc.tile_pool(name=f"xt{r}", bufs=4 * n_d))
    pool_xq = ctx.enter_context(tc.tile_pool(name=f"xq{r}", bufs=n_d))
    pool_xn = ctx.enter_context(tc.tile_pool(name=f"xn{r}", bufs=n_j))
    pool_a = ctx.enter_context(tc.tile_pool(name=f"a{r}", bufs=n_d))
    pool_wv = ctx.enter_context(tc.tile_pool(name=f"wv{r}", bufs=n_d))
    pool_tt = ctx.enter_context(tc.tile_pool(name=f"tt{r}", bufs=n_d))
    pool_mask = ctx.enter_context(tc.tile_pool(name=f"mask{r}", bufs=16))
    pool_probs = ctx.enter_context(tc.tile_pool(name=f"probs{r}", bufs=16))
    pool_ut = ctx.enter_context(tc.tile_pool(name=f"ut{r}", bufs=2 * n_d))
    pool_lr = ctx.enter_context(tc.tile_pool(name=f"lr{r}", bufs=4))
    pool_out = ctx.enter_context(tc.tile_pool(name=f"outb{r}", bufs=2))
    pool_one = ctx.enter_context(tc.tile_pool(name=f"one{r}", bufs=1))

    # ---- loads ----
    ats = []
    for dt in range(n_d):
        t = pool_a.tile([P, D], MDT, tag="a", name=f"at{r}_{dt}")
        nc.scalar.dma_start(out=t, in_=a_d[dt * P:(dt + 1) * P, :])
        ats.append(t)
    xqs = []
    for dt in range(n_d):
        t = pool_xq.tile([P, NQ], MDT, tag="xq", name=f"xqt{r}_{dt}")
        nc.sync.dma_start(out=t, in_=xt_d[dt * P:(dt + 1) * P, N:])
        xqs.append(t)
    # key tiles [dt][jc]: [128, 512] each (4 j-chunks); DMA in first-use order
    xtk = [[None] * 4 for _ in range(n_d)]
    xns = [None] * n_j
    mask_tiles = [[None] * 4 for _ in range(4)]
    wvs = [None] * n_d

    def load_keys(jc):
        for dt in range(n_d):
            t = pool_xt.tile([P, 512], MDT, tag="xt", name=f"xtt{r}_{dt}_{jc}")
            nc.sync.dma_start(out=t, in_=xt_d[dt * P:(dt + 1) * P,
                                             jc * 512:(jc + 1) * 512])
            xtk[dt][jc] = t

    def load_xn(jc):
        for jt in range(4 * jc, 4 * jc + 4):
            t = pool_xn.tile([P, D], MDT, tag="xn", name=f"xnt{r}_{jt}")
            nc.sync.dma_start(out=t, in_=xn_d[jt * P:(jt + 1) * P, :])
            xns[jt] = t

    load_keys(0)
    load_xn(0)
    for s in range(4):
        for k in range(4):
            t = pool_mask.tile([P, IB], MDT, tag="mask",
                               name=f"mask{r}_{s}_{k}")
            nc.scalar.dma_start(out=t, in_=mask_d[s, k, :, :])
            mask_tiles[s][k] = t
    for dt in range(n_d):
        t = pool_wv.tile([P, D], MDT, tag="wv", name=f"wvt{r}_{dt}")
        nc.scalar.dma_start(out=t, in_=wvt_d[dt * P:(dt + 1) * P, :])
        wvs[dt] = t
    for jc in range(1, 4):
        load_keys(jc)
        load_xn(jc)
    ones = pool_one.tile([P, 1], MDT, tag="one", name=f"ones{r}")
    nc.vector.memset(ones, 1.0)

    # ---- phase 1: T^T[d2, i] = sum_d1 A[d1, d2] x^T[d1, i] over own queries
    tts = [pool_tt.tile([P, NQ], MDT, tag="tt", name=f"ttt{r}_{i}")
           for i in range(n_d)]
    with tc.tile_pool(name=f"ps1_{r}", bufs=8, space="PSUM") as ps1:
        for c0 in range(0, NQ, IB):
            for d2 in range(n_d):
                ps = ps1.tile([P, IB], mybir.dt.float32, tag="ps1",
                              name=f"pst{r}_{d2}_{c0}")
                for d1 in range(n_d):
                    nc.tensor.matmul(
                        ps,
                        lhsT=ats[d1][:, d2 * P:(d2 + 1) * P],
                        rhs=xqs[d1][:, c0:c0 + IB],
                        start=(d1 == 0), stop=(d1 == n_d - 1),
                    )
                nc.vector.tensor_copy(tts[d2][:, c0:c0 + IB], ps)

    # ---- phase 2: attention ----
    with (
        tc.tile_pool(name=f"ps_s{r}", bufs=3, space="PSUM") as ps_s,
        tc.tile_pool(name=f"ps_u{r}", bufs=2, space="PSUM") as ps_u,
        tc.tile_pool(name=f"ps_f{r}", bufs=2, space="PSUM") as ps_f,
        tc.tile_pool(name=f"ps_l{r}", bufs=1, space="PSUM") as ps_l,
    ):
        for s in range(4):
            trips = TRIPS[s]
            # pass 1: scores + exp; probs tiles persist for the slot
            probs_tiles = []
            for jt in range(trips):
                pss = ps_s.tile([P, IB], mybir.dt.float32, tag="pss",
                                name=f"pss{r}_{s}_{jt}")
                for d2 in range(n_d):
                    nc.tensor.matmul(
                        pss,
                        lhsT=xtk[d2][jt // 4][:, (jt % 4) * P:(jt % 4 + 1) * P],
                        rhs=tts[d2][:, s * IB:(s + 1) * IB],
                        start=(d2 == 0), stop=(d2 == n_d - 1),
                    )
                k = jt - (trips - 4)
                if k >= 0:
                    nc.vector.tensor_add(pss, pss, mask_tiles[s][k])
                probs = pool_probs.tile([P, IB], MDT, tag="probs",
                                        name=f"probs{r}_{s}_{jt}")
                nc.scalar.activation(probs, pss,
                                     mybir.ActivationFunctionType.Exp,
                                     scale=SCALE)
                probs_tiles.append(probs)

            # pass 2: U^T[d-tile, i] = sum_jt xn_tile.T @ probs, one bank
            # at a time so each accumulation group owns its bank exclusively.
            # The l^T (softmax denominator) matmuls are interleaved so their
            # LDWEIGHTS hide under the U streams in the PE reorder window.
            psl2 = ps_l.tile([P, 2], mybir.dt.float32, tag="l",
                             name=f"psl{r}_{s}")
            psl = [psl2[:, 0:1], psl2[:, 1:2]]
            uts = []
            for dt in range(n_d):
                psu = ps_u.tile([P, IB], mybir.dt.float32, tag="u",
                                name=f"psu{r}_{s}_{dt}")
                for jt in range(trips):
                    nc.tensor.matmul(
                        psu,
                        lhsT=xns[jt][:, dt * P:(dt + 1) * P],
                        rhs=probs_tiles[jt],
                        start=(jt == 0), stop=(jt == trips - 1),
                    )
                    if dt < 2:
                        nc.tensor.matmul(
                            psl[dt],
                            lhsT=probs_tiles[jt][:, dt * P:(dt + 1) * P],
                            rhs=ones,
                            start=(jt == 0 and dt == 0),
                            stop=(jt == trips - 1),
                        )
                ut = pool_ut.tile([P, IB], MDT, tag="ut",
                                  name=f"ut{r}_{s}_{dt}")
                nc.vector.tensor_copy(ut, psu)
                uts.append(ut)

            # out[i, o'] = (sum_d U^T[d, i]^T Wv^T[d, o']) / l[i]
            for half in range(2):
                rt = pool_lr.tile([P, 1], mybir.dt.float32, tag="lr",
                                  name=f"lrec{r}_{s}_{half}")
                nc.vector.reciprocal(rt, psl[half])
                obh = pool_out.tile([P, D], mybir.dt.float32, tag="obh",
                                    name=f"obh{r}_{s}_{half}")
                for c0 in range(0, D, 512):
                    psf = ps_f.tile([P, 512], mybir.dt.float32, tag="f",
                                    name=f"psf{r}_{s}_{half}_{c0}")
                    for dt in range(n_d):
                        nc.tensor.matmul(
                            psf,
                            lhsT=uts[dt][:, half * P:(half + 1) * P],
                            rhs=wvs[dt][:, c0:c0 + 512],
                            start=(dt == 0), stop=(dt == n_d - 1),
                        )
                    nc.vector.tensor_scalar_mul(obh[:, c0:c0 + 512], psf, rt)
                    r0 = s * IB + half * P
                    nc.sync.dma_start(out=out_d[r0:r0 + P, c0:c0 + 512],
                                      in_=obh[:, c0:c0 + 512])


def _build_nc(reps=1):
    nc = bacc.Bacc(None, target_bir_lowering=False)
    P = 128
    xt_w = N + NQ

    xt_d = nc.declare_dram_parameter("xt", [D, xt_w], MDT, isOutput=False)
    xn_d = nc.declare_dram_parameter("xn", [N, D], MDT, isOutput=False)
    a_d = nc.declare_dram_parameter("a", [D, D], MDT, isOutput=False)
    wvt_d = nc.declare_dram_parameter("wvt", [D, D], MDT, isOutput=False)
    mask_d = nc.declare_dram_parameter("masks", [4, 4, P, IB], MDT,
                                       isOutput=False)
    out_d = nc.declare_dram_parameter("out_p", [NQ, D], mybir.dt.float32,
                                      isOutput=True)
    dram = (xt_d, xn_d, a_d, wvt_d, mask_d, out_d)

    with tile.TileContext(nc) as tc:
        for rep in range(reps):
            with ExitStack() as ctx:
                _build_body(nc, tc, ctx, dram, rep)
    nc.finalize()
    return nc


def _make_in_maps(x, W_q, W_k, W_v):
    wq = np.asarray(W_q, np.float32)
    wk = np.asarray(W_k, np.float32)
    wv = np.asarray(W_v, np.float32)
    a = (wq.T @ wk).astype(NPDT)                       # [d1, d2]
    wvt = np.ascontiguousarray(wv.T).astype(NPDT)      # [d, o]
    masks = [_build_masks(0), _build_masks(1)]
    qrows = [_qrows(0), _qrows(1)]
    in_maps = []
    for c in range(NCORES):
        b, h = c // 2, c % 2
        xb = x[b]
        xb_t = xb.T  # [D, N]
        xt_all = np.concatenate([xb_t, xb_t[:, qrows[h]]], axis=1)
        in_maps.append({
            "xt": np.ascontiguousarray(xt_all).astype(NPDT),
            "xn": xb.astype(NPDT),
            "a": a, "wvt": wvt,
            "masks": masks[h].astype(NPDT),
        })
    return in_maps


def kernel(x, W_q, W_k, W_v):
    global _CACHED_NC, LAST_RESULT, LAST_IN_MAPS
    x = np.asarray(x, dtype=np.float32)
    if _CACHED_NC is None:
        _CACHED_NC = _build_nc()
    nc = _CACHED_NC

    in_maps = _make_in_maps(x, W_q, W_k, W_v)
    LAST_IN_MAPS = in_maps
    try:
        res = run_bass_kernel_spmd(nc, in_maps, list(range(NCORES)))
    except Exception:
        # transient NRT_EXEC_UNIT_UNRECOVERABLE wedges clear on retry
        import time as _time
        _time.sleep(5)
        res = run_bass_kernel_spmd(nc, in_maps, list(range(NCORES)))
    LAST_RESULT = res

    qrows = [_qrows(0), _qrows(1)]
    out = np.empty((B, N, D), np.float32)
    for c in range(NCORES):
        b, h = c // 2, c % 2
        out[b, qrows[h], :] = res.results[c]["out_p"]
    return out



# revision 2
# speedup vs baseline: 1.0891x; 1.0891x over previous
"""Causal single-head attention on 8 trn2 NeuronCores — fp8 DoubleRow version.

Sharding: core c handles batch c//2, half h = c%2 of the query rows (4 blocks
of 256 rows picked so causal work balances). Identical device program on all
cores; row ownership is data (host-side gather/scatter + per-core masks).

Algorithm — projections folded, all matmuls fp8 e4m3 hi/lo 3-combo DoubleRow:
  host:  A32 = 32 * Wq^T Wk, Wv32 = 32 * Wv^T   (scaled so fp8 hi/lo splits
         stay out of e4m3's subnormal range), x split to (hi, lo) e4m3 pairs
         in the DoubleRow layouts (contraction pairs packed on the free dim).
  dev:   T^T = A32^T x^T over own queries      -> quantize psum to (Th, Tl)
         S^T = x^T.T T^T  (per query slot)     -> +mask, e = exp(S/1024)
         probs = (ph, pl) e4m3                  (hi copy + residual)
         U^T  = (x/2)^T probs                  -> quantize to (Uh, Ul)
         l16  = ones16^T probs                  (DoubleRow, 1-col)
         out  = (U^T.T Wv32^T) * recip(l16)    -> bf16, host upcasts.
  Each matmul group contracts (hi*hi + hi*lo + lo*hi): bf16-level accuracy at
  1/4 bf16 PE cost per combo (DoubleRow = 2 k-tiles/instr at 0.5 cyc/row).
"""

import sys

try:
    import concourse  # noqa: F401
except ImportError:
    sys.path.insert(0, "/opt/trn_rl_repo")

from contextlib import ExitStack

import ml_dtypes
import numpy as np

import concourse.bass as bass
from concourse import bacc
import concourse.mybir as mybir
import concourse.tile as tile
from concourse.bass_utils import run_bass_kernel_spmd

B, N, D = 4, 2048, 1024
NQ = 1024
NCORES = 8
TRIPS = (4, 8, 12, 16)
SLOTS = ((0, 2, 4, 6), (1, 3, 5, 7))
IB = 256
SCALE = 1.0 / 1024.0     # exp scale: 1/sqrt(D) / 32 (A32 carries 32x)
P = 128

F8 = mybir.dt.float8e4
F32 = mybir.dt.float32
BF16 = mybir.dt.bfloat16
DRM = mybir.MatmulPerfMode.DoubleRow
E4NP = ml_dtypes.float8_e4m3
BFNP = ml_dtypes.bfloat16

TRACE = False
LAST_RESULT = None
LAST_IN_MAPS = None
_CACHED_NC = None


def _qrows(h):
    return np.concatenate([np.arange(256 * p, 256 * p + 256) for p in SLOTS[h]])


def _build_masks(h):
    """[4 slots, 4, 128, 256] f32 additive causal masks, last 4 j-tiles."""
    masks = np.zeros((4, 4, P, IB), np.float32)
    jp = np.arange(P)[:, None]
    iv = np.arange(IB)[None, :]
    for s in range(4):
        r0 = 256 * SLOTS[h][s]
        for k in range(4):
            jt = TRIPS[s] - 4 + k
            masks[s, k] = np.where(jt * P + jp <= r0 + iv, 0.0, -1e30)
    return masks


def _mm(nc, out, lhsT, rhs, start, stop):
    nc.tensor.matmul(out, lhsT=lhsT, rhs=rhs, start=start, stop=stop,
                     perf_mode=DRM)


def _build_body(nc, tc, ctx, dram):
    (a_d, xq_d, xk_d, xn_d, wv_d, mask_d, out_d) = dram

    pool_a = ctx.enter_context(tc.tile_pool(name="a", bufs=8))
    pool_xq = ctx.enter_context(tc.tile_pool(name="xq", bufs=8))
    pool_xk = ctx.enter_context(tc.tile_pool(name="xk", bufs=2))
    pool_xn = ctx.enter_context(tc.tile_pool(name="xn", bufs=2))
    pool_wv = ctx.enter_context(tc.tile_pool(name="wv", bufs=2))
    pool_mask = ctx.enter_context(tc.tile_pool(name="mask", bufs=1))
    pool_t = ctx.enter_context(tc.tile_pool(name="tq", bufs=8))
    pool_e = ctx.enter_context(tc.tile_pool(name="e32", bufs=4))
    pool_p = ctx.enter_context(tc.tile_pool(name="probs", bufs=28))
    pool_u = ctx.enter_context(tc.tile_pool(name="uq", bufs=16))
    pool_ob = ctx.enter_context(tc.tile_pool(name="ob", bufs=4))
    pool_rt = ctx.enter_context(tc.tile_pool(name="rt", bufs=4))
    pool_one = ctx.enter_context(tc.tile_pool(name="one", bufs=1))

    # ---- loads (order = DMA priority: T operands, then scores, U, out) ----
    at = [[None] * 4 for _ in range(2)]   # [hl][pp] -> [128, 2, 8, 128]
    xqt = [[None] * 4 for _ in range(2)]  # [hl][pp] -> [128, 2, 1024]
    for pp in range(4):
        for hl in range(2):
            t = pool_a.tile([P, 2, 8, P], F8, tag="a", name=f"a{hl}_{pp}")
            nc.sync.dma_start(out=t, in_=a_d[hl, pp, :, :, :, :])
            at[hl][pp] = t
            t = pool_xq.tile([P, 2, NQ], F8, tag="xq", name=f"xq{hl}_{pp}")
            nc.sync.dma_start(out=t, in_=xq_d[hl, pp, :, :, :])
            xqt[hl][pp] = t
    xkt = []                              # [hl] -> [128, 4, 2, 2048]
    for hl in range(2):
        t = pool_xk.tile([P, 4, 2, N], F8, tag="xk", name=f"xk{hl}")
        nc.sync.dma_start(out=t, in_=xk_d[hl, :, :, :, :])
        xkt.append(t)
    maskt = pool_mask.tile([P, 4, 4, IB], BF16, tag="mask", name="maskt")
    nc.sync.dma_start(out=maskt, in_=mask_d[:, :, :, :])
    xnt = []                              # [hl] -> [128, 8, 2, 8, 128]
    for hl in range(2):
        t = pool_xn.tile([P, 8, 2, 8, P], F8, tag="xn", name=f"xn{hl}")
        nc.sync.dma_start(out=t, in_=xn_d[hl, :, :, :, :, :])
        xnt.append(t)
    wvt = []                              # [hl] -> [128, 4, 2, 1024]
    for hl in range(2):
        t = pool_wv.tile([P, 4, 2, NQ], F8, tag="wv", name=f"wv{hl}")
        nc.sync.dma_start(out=t, in_=wv_d[hl, :, :, :, :])
        wvt.append(t)
    ones = pool_one.tile([P, 2, 1], F8, tag="one", name="ones")
    nc.vector.memset(ones, 16.0)

    # T^T pair tiles: [dp] -> [128, 2, 1024] (i = d2-tile parity)
    tht = [pool_t.tile([P, 2, NQ], F8, tag="tq", name=f"th{dp}")
           for dp in range(4)]
    tlt = [pool_t.tile([P, 2, NQ], F8, tag="tq", name=f"tl{dp}")
           for dp in range(4)]

    # ---- phase 1: T^T[d2, q] = sum_d1 A32[d1, d2] x^T[d1, q] ----
    with tc.tile_pool(name="pt", bufs=2, space="PSUM") as pt_pool:
        for qb in range(4):
            for d2t in range(8):
                pt = pt_pool.tile([P, IB], F32, tag="pt",
                                  name=f"pt{qb}_{d2t}")
                k = 0
                for ha, hx in ((0, 0), (0, 1), (1, 0)):
                    for pp in range(4):
                        _mm(nc, pt,
                            at[ha][pp][:, :, d2t, :],
                            xqt[hx][pp][:, :, qb * IB:(qb + 1) * IB],
                            start=(k == 0), stop=(k == 11))
                        k += 1
                hs = tht[d2t // 2][:, d2t % 2, qb * IB:(qb + 1) * IB]
                ls = tlt[d2t // 2][:, d2t % 2, qb * IB:(qb + 1) * IB]
                nc.scalar.activation(hs, pt,
                                     mybir.ActivationFunctionType.Copy)
                nc.vector.tensor_sub(ls, pt, hs)

    # ---- phase 2 ----
    with (
        tc.tile_pool(name="pss", bufs=3, space="PSUM") as ps_s,
        tc.tile_pool(name="psu", bufs=2, space="PSUM") as ps_u,
        tc.tile_pool(name="psf", bufs=2, space="PSUM") as ps_f,
        tc.tile_pool(name="psl", bufs=1, space="PSUM") as ps_l,
    ):
        probs = [None] * 4   # per slot: (ph list, pl list) by key-pair
        psls = [None] * 4
        rts = [None] * 4
        uqs = [None] * 4     # per slot: (uh list, ul list) by d-pair

        def emit_scores(s):
            pairs = TRIPS[s] // 2
            ph_t = [pool_p.tile([P, 2, IB], F8, tag="probs",
                                name=f"ph{s}_{j}") for j in range(pairs)]
            pl_t = [pool_p.tile([P, 2, IB], F8, tag="probs",
                                name=f"pl{s}_{j}") for j in range(pairs)]
            probs[s] = (ph_t, pl_t)
            for jt in range(TRIPS[s]):
                pss = ps_s.tile([P, IB], F32, tag="pss", name=f"pss{s}_{jt}")
                k = 0
                for hx, ht in ((0, 0), (0, 1), (1, 0)):
                    tt = tht if ht == 0 else tlt
                    for dp in range(4):
                        _mm(nc, pss,
                            xkt[hx][:, dp, :, jt * P:(jt + 1) * P],
                            tt[dp][:, :, s * IB:(s + 1) * IB],
                            start=(k == 0), stop=(k == 11))
                        k += 1
                kk = jt - (TRIPS[s] - 4)
                if kk >= 0:
                    nc.vector.tensor_add(pss, pss, maskt[:, s, kk, :])
                e = pool_e.tile([P, IB], F32, tag="e32", name=f"e{s}_{jt}")
                nc.scalar.activation(e, pss,
                                     mybir.ActivationFunctionType.Exp,
                                     scale=SCALE)
                hp = ph_t[jt // 2][:, jt % 2, :]
                lp = pl_t[jt // 2][:, jt % 2, :]
                nc.vector.tensor_copy(hp, e)
                nc.vector.tensor_sub(lp, e, hp)

        def emit_u(s):
            pairs = TRIPS[s] // 2
            ph_t, pl_t = probs[s]
            # softmax denominator (x16): 1-col DoubleRow matmuls
            psl = ps_l.tile([P, 2], F32, tag="psl", name=f"psl{s}")
            psls[s] = psl
            k = 0
            last = 2 * 2 * pairs - 1
            for hf in range(2):
                for j in range(pairs):
                    for t in (ph_t, pl_t):
                        _mm(nc, psl[:, hf:hf + 1],
                            t[j][:, :, hf * P:(hf + 1) * P], ones,
                            start=(k == 0), stop=(k == last))
                        k += 1
            uh_t = [pool_u.tile([P, 2, IB], F8, tag="uq",
                                name=f"uh{s}_{d}") for d in range(4)]
            ul_t = [pool_u.tile([P, 2, IB], F8, tag="uq",
                                name=f"ul{s}_{d}") for d in range(4)]
            uqs[s] = (uh_t, ul_t)
            for dt in range(8):
                psu = ps_u.tile([P, IB], F32, tag="psu", name=f"psu{s}_{dt}")
                k = 0
                for hx, hp in ((0, 0), (1, 0), (0, 1)):
                    pt_ = ph_t if hp == 0 else pl_t
                    for j in range(pairs):
                        _mm(nc, psu,
                            xnt[hx][:, j, :, dt, :], pt_[j],
                            start=(k == 0), stop=(k == 3 * pairs - 1))
                        k += 1
                hs = uh_t[dt // 2][:, dt % 2, :]
                ls = ul_t[dt // 2][:, dt % 2, :]
                nc.scalar.activation(hs, psu,
                                     mybir.ActivationFunctionType.Copy)
                nc.vector.tensor_sub(ls, psu, hs)
            rt = pool_rt.tile([P, 2], F32, tag="rt", name=f"rt{s}")
            nc.vector.reciprocal(rt, psl)
            rts[s] = rt

        def emit_out(s):
            uh_t, ul_t = uqs[s]
            for hf in range(2):
                for ob in range(4):
                    psf = ps_f.tile([P, IB], F32, tag="psf",
                                    name=f"psf{s}_{hf}_{ob}")
                    k = 0
                    for hu, hw in ((0, 0), (0, 1), (1, 0)):
                        ut = uh_t if hu == 0 else ul_t
                        for dp in range(4):
                            _mm(nc, psf,
                                ut[dp][:, :, hf * P:(hf + 1) * P],
                                wvt[hw][:, dp, :, ob * IB:(ob + 1) * IB],
                                start=(k == 0), stop=(k == 11))
                            k += 1
                    ob_t = pool_ob.tile([P, IB], BF16, tag="ob",
                                        name=f"ob{s}_{hf}_{ob}")
                    nc.vector.tensor_scalar_mul(ob_t, psf,
                                                rts[s][:, hf:hf + 1])
                    r0 = s * IB + hf * P
                    nc.sync.dma_start(
                        out=out_d[r0:r0 + P, ob * IB:(ob + 1) * IB],
                        in_=ob_t)

        emit_scores(0)
        emit_scores(1)
        emit_u(0)
        emit_scores(2)
        emit_u(1)
        emit_out(0)
        emit_scores(3)
        emit_u(2)
        emit_out(1)
        emit_u(3)
        emit_out(2)
        emit_out(3)


def _build_nc():
    nc = bacc.Bacc(None, target_bir_lowering=False)
    a_d = nc.declare_dram_parameter("a8", [2, 4, P, 2, 8, P], F8,
                                    isOutput=False)
    xq_d = nc.declare_dram_parameter("xq8", [2, 4, P, 2, NQ], F8,
                                     isOutput=False)
    xk_d = nc.declare_dram_parameter("xk8", [2, P, 4, 2, N], F8,
                                     isOutput=False)
    xn_d = nc.declare_dram_parameter("xn8", [2, P, 8, 2, 8, P], F8,
                                     isOutput=False)
    wv_d = nc.declare_dram_parameter("wv8", [2, P, 4, 2, NQ], F8,
                                     isOutput=False)
    mask_d = nc.declare_dram_parameter("masks", [P, 4, 4, IB], BF16,
                                       isOutput=False)
    out_d = nc.declare_dram_parameter("out_p", [NQ, D], BF16, isOutput=True)
    dram = (a_d, xq_d, xk_d, xn_d, wv_d, mask_d, out_d)

    with tile.TileContext(nc) as tc:
        with ExitStack() as ctx:
            _build_body(nc, tc, ctx, dram)
    nc.finalize()
    return nc


def _split8(a):
    h = a.astype(E4NP)
    l = (a - h.astype(np.float32)).astype(E4NP)
    return h, l


def _pack_pairs(arr, inner):
    """[K, M] -> [K//256 pairs, 128, 2, *inner] with K = pp*256 + i*128 + p."""
    k = arr.shape[0]
    return np.ascontiguousarray(
        arr.reshape(k // 256, 2, P, *inner).transpose(0, 2, 1, *range(3, 3 + len(inner))))


def _make_in_maps(x, W_q, W_k, W_v):
    wq = np.asarray(W_q, np.float32)
    wk = np.asarray(W_k, np.float32)
    wv = np.asarray(W_v, np.float32)
    A32 = (wq.T.astype(np.float64) @ wk.astype(np.float64) * 32.0).astype(np.float32)
    Wvt32 = np.ascontiguousarray(wv.T) * 32.0

    # a8: [2, 4 pp, 128, 2, 8, 128]
    def lhs_pack(m):       # [d1, d2] -> [4, 128, 2, 8, 128]
        return m.reshape(4, 2, P, 8, P).transpose(0, 2, 1, 3, 4)
    ah, al = _split8(A32)
    a8 = np.ascontiguousarray(np.stack([lhs_pack(ah), lhs_pack(al)]))
    # wv8: [2, 128, 4 dp, 2, 1024]
    def rhs_pack(m):       # [d, o] -> [128, 4, 2, 1024]
        return m.reshape(4, 2, P, NQ).transpose(2, 0, 1, 3)
    wh, wl = _split8(Wvt32)
    wv8 = np.ascontiguousarray(np.stack([rhs_pack(wh), rhs_pack(wl)]))

    masks = [None, None]
    for h in range(2):
        mk = _build_masks(h).astype(BFNP)       # [4, 4, 128, 256]
        masks[h] = np.ascontiguousarray(mk.transpose(2, 0, 1, 3))
    qrows = [_qrows(0), _qrows(1)]

    in_maps = []
    per_batch = {}
    for b in range(B):
        xb = np.asarray(x[b], np.float32)
        xh, xl = _split8(xb)                    # [2048, 1024] e4m3
        xh32 = xh.astype(np.float32)
        xl32 = xl.astype(np.float32)
        # xk8: [2, 128, 4 dp, 2, 2048]: from x^T [1024 d, 2048 k]
        def xk_pack(m32):
            return m32.T.reshape(4, 2, P, N).transpose(2, 0, 1, 3)
        xk8 = np.ascontiguousarray(np.stack(
            [xk_pack(xh32), xk_pack(xl32)])).astype(E4NP)
        # xn8: [2, 128, 8 kp, 2, 8 dt, 128]: from x/2 [2048 k, 1024 d]
        def xn_pack(m32):
            return (m32 * 0.5).reshape(8, 2, P, 8, P).transpose(2, 0, 1, 3, 4)
        xn8 = np.ascontiguousarray(np.stack(
            [xn_pack(xh32), xn_pack(xl32)])).astype(E4NP)
        per_batch[b] = (xh32, xl32, xk8, xn8)

    for c in range(NCORES):
        b, h = c // 2, c % 2
        xh32, xl32, xk8, xn8 = per_batch[b]
        # xq8: [2, 4 pp, 128, 2, 1024]: x^T[:, qrows]
        def xq_pack(m32):
            return m32.T[:, qrows[h]].reshape(4, 2, P, NQ).transpose(0, 2, 1, 3)
        xq8 = np.ascontiguousarray(np.stack(
            [xq_pack(xh32), xq_pack(xl32)])).astype(E4NP)
        in_maps.append({
            "a8": a8, "xq8": xq8, "xk8": xk8, "xn8": xn8,
            "wv8": wv8, "masks": masks[h],
        })
    return in_maps


def kernel(x, W_q, W_k, W_v):
    global _CACHED_NC, LAST_RESULT, LAST_IN_MAPS
    x = np.asarray(x, dtype=np.float32)
    if _CACHED_NC is None:
        _CACHED_NC = _build_nc()
    nc = _CACHED_NC

    in_maps = _make_in_maps(x, W_q, W_k, W_v)
    LAST_IN_MAPS = in_maps
    try:
        res = run_bass_kernel_spmd(nc, in_maps, list(range(NCORES)))
    except Exception:
        # transient NRT_EXEC_UNIT_UNRECOVERABLE wedges clear on retry
        import time as _time
        _time.sleep(5)
        res = run_bass_kernel_spmd(nc, in_maps, list(range(NCORES)))
    LAST_RESULT = res

    qrows = [_qrows(0), _qrows(1)]
    out = np.empty((B, N, D), np.float32)
    for c in range(NCORES):
        b, h = c // 2, c % 2
        out[b, qrows[h], :] = np.asarray(res.results[c]["out_p"],
                                         dtype=np.float32)
    return out


# revision 4
# speedup vs baseline: 1.1437x; 1.0502x over previous
"""Causal single-head attention on 8 trn2 NeuronCores — fp8 DoubleRow version.

Sharding: core c handles batch c//2, half h = c%2 of the query rows (4 blocks
of 256 rows picked so causal work balances). Identical device program on all
cores; row ownership is data (host-side gather/scatter + per-core masks).

Algorithm — projections folded, all matmuls fp8 e4m3 hi/lo 3-combo DoubleRow:
  host:  A32 = 32 * Wq^T Wk, Wv32 = 32 * Wv^T   (scaled so fp8 hi/lo splits
         stay out of e4m3's subnormal range), x split to (hi, lo) e4m3 pairs
         in the DoubleRow layouts (contraction pairs packed on the free dim).
  dev:   T^T = A32^T x^T over own queries      -> quantize psum to (Th, Tl)
         S^T = x^T.T T^T  (per query slot)     -> +mask, e = exp(S/1024)
         probs = (ph, pl) e4m3                  (hi copy + residual)
         U^T  = (x/2)^T probs                  -> quantize to (Uh, Ul)
         l16  = ones16^T probs                  (DoubleRow, 1-col)
         out  = (U^T.T Wv32^T) * recip(l16)    -> bf16, host upcasts.
  Each matmul group contracts (hi*hi + hi*lo + lo*hi): bf16-level accuracy at
  1/4 bf16 PE cost per combo (DoubleRow = 2 k-tiles/instr at 0.5 cyc/row).

Scheduling: one shared 7-deep PSUM ring for every [128,256] accumulation
group so ring reuse follows emission order; T(qb) interleaved with the slot
pipeline; DMAs chunked (xq per query-block, xn per key-pair, T tiles per-qb)
so consumers wait only on the chunks they read.
"""

import sys

try:
    import concourse  # noqa: F401
except ImportError:
    sys.path.insert(0, "/opt/trn_rl_repo")

from contextlib import ExitStack

import ml_dtypes
import numpy as np

import concourse.bass as bass
from concourse import bacc
import concourse.mybir as mybir
import concourse.tile as tile
from concourse.bass_utils import run_bass_kernel_spmd

B, N, D = 4, 2048, 1024
NQ = 1024
NCORES = 8
TRIPS = (4, 8, 12, 16)
SLOTS = ((0, 2, 4, 6), (1, 3, 5, 7))
IB = 256
SCALE = 1.0 / 1024.0     # exp scale: 1/sqrt(D) / 32 (A32 carries 32x)
P = 128

F8 = mybir.dt.float8e4
F32 = mybir.dt.float32
BF16 = mybir.dt.bfloat16
DRM = mybir.MatmulPerfMode.DoubleRow
E4NP = ml_dtypes.float8_e4m3
BFNP = ml_dtypes.bfloat16

TRACE = False
LAST_RESULT = None
LAST_IN_MAPS = None
_CACHED_NC = None


def _qrows(h):
    return np.concatenate([np.arange(256 * p, 256 * p + 256) for p in SLOTS[h]])


def _build_masks(h):
    """[4 slots, 4, 128, 256] f32 additive causal masks, last 4 j-tiles."""
    masks = np.zeros((4, 4, P, IB), np.float32)
    jp = np.arange(P)[:, None]
    iv = np.arange(IB)[None, :]
    for s in range(4):
        r0 = 256 * SLOTS[h][s]
        for k in range(4):
            jt = TRIPS[s] - 4 + k
            masks[s, k] = np.where(jt * P + jp <= r0 + iv, 0.0, -1e30)
    return masks


def _mm(nc, out, lhsT, rhs, start, stop):
    nc.tensor.matmul(out, lhsT=lhsT, rhs=rhs, start=start, stop=stop,
                     perf_mode=DRM)


def _build_body(nc, tc, ctx, dram):
    (a_d, xq_d, xk_d, xn_d, wv_d, mask_d, out_d) = dram

    pool_a = ctx.enter_context(tc.tile_pool(name="a", bufs=8))
    pool_xq = ctx.enter_context(tc.tile_pool(name="xq", bufs=32))
    pool_xk = ctx.enter_context(tc.tile_pool(name="xk", bufs=2))
    pool_xn = ctx.enter_context(tc.tile_pool(name="xn", bufs=16))
    pool_wv = ctx.enter_context(tc.tile_pool(name="wv", bufs=2))
    pool_mask = ctx.enter_context(tc.tile_pool(name="mask", bufs=1))
    pool_t = ctx.enter_context(tc.tile_pool(name="tq", bufs=32))
    pool_e = ctx.enter_context(tc.tile_pool(name="e32", bufs=4))
    pool_p = ctx.enter_context(tc.tile_pool(name="probs", bufs=28))
    pool_u = ctx.enter_context(tc.tile_pool(name="uq", bufs=16))
    pool_ob = ctx.enter_context(tc.tile_pool(name="ob", bufs=4))
    pool_rt = ctx.enter_context(tc.tile_pool(name="rt", bufs=4))
    pool_one = ctx.enter_context(tc.tile_pool(name="one", bufs=1))

    # ---- DMA issue stream (single SP sequencer, priority order) ----
    at = [[None] * 4 for _ in range(2)]    # [hl][pp] -> [128, 2, 8, 128]
    xqt = [[[None] * 4 for _ in range(4)] for _ in range(2)]  # [hl][qb][pp]
    xkt = [None] * 2                       # [hl] -> [128, 4, 2, 2048]
    xnt = [[None] * 8 for _ in range(2)]   # [hl][kp] -> [128, 2, 8, 128]
    wvt = [None] * 2                       # [hl] -> [128, 4, 2, 1024]

    def load_a():
        for pp in range(4):
            for hl in range(2):
                t = pool_a.tile([P, 2, 8, P], F8, tag="a", name=f"a{hl}_{pp}")
                nc.sync.dma_start(out=t, in_=a_d[hl, pp, :, :, :, :])
                at[hl][pp] = t

    def load_xq(qb):
        for pp in range(4):
            for hl in range(2):
                t = pool_xq.tile([P, 2, IB], F8, tag="xq",
                                 name=f"xq{hl}_{qb}_{pp}")
                nc.sync.dma_start(out=t, in_=xq_d[hl, qb, pp, :, :, :])
                xqt[hl][qb][pp] = t

    def load_xk(hl):
        t = pool_xk.tile([P, 4, 2, N], F8, tag="xk", name=f"xk{hl}")
        nc.sync.dma_start(out=t, in_=xk_d[hl, :, :, :, :])
        xkt[hl] = t

    def load_xn(kp0, kp1):
        for kp in range(kp0, kp1):
            for hl in range(2):
                t = pool_xn.tile([P, 2, 8, P], F8, tag="xn",
                                 name=f"xn{hl}_{kp}")
                nc.sync.dma_start(out=t, in_=xn_d[hl, kp, :, :, :, :])
                xnt[hl][kp] = t

    def load_wv(hl):
        t = pool_wv.tile([P, 4, 2, NQ], F8, tag="wv", name=f"wv{hl}")
        nc.sync.dma_start(out=t, in_=wv_d[hl, :, :, :, :])
        wvt[hl] = t

    load_a()
    load_xq(0)
    load_xq(1)
    load_xk(0)
    load_xk(1)
    load_xq(2)
    load_xq(3)
    maskt = pool_mask.tile([P, 4, 4, IB], BF16, tag="mask", name="maskt")
    nc.sync.dma_start(out=maskt, in_=mask_d[:, :, :, :])
    load_xn(0, 4)
    load_wv(0)
    load_xn(4, 8)
    load_wv(1)
    ones = pool_one.tile([P, 2, 1], F8, tag="one", name="ones")
    nc.vector.memset(ones, 16.0)

    # T^T tiles per (qb, dp): [128, 2, 256] (dim1 = d2-tile parity)
    tht = [[pool_t.tile([P, 2, IB], F8, tag="tq", name=f"th{qb}_{dp}")
            for dp in range(4)] for qb in range(4)]
    tlt = [[pool_t.tile([P, 2, IB], F8, tag="tq", name=f"tl{qb}_{dp}")
            for dp in range(4)] for qb in range(4)]

    with (
        tc.tile_pool(name="psA", bufs=7, space="PSUM") as ps_a,
        tc.tile_pool(name="psl", bufs=1, space="PSUM") as ps_l,
    ):
        probs = [None] * 4   # per slot: (ph list, pl list) by key-pair
        psls = [None] * 4
        rts = [None] * 4
        uqs = [None] * 4     # per slot: (uh list, ul list) by d-pair

        def emit_t(qb):
            for d2t in range(8):
                pt = ps_a.tile([P, IB], F32, tag="ps", name=f"pt{qb}_{d2t}")
                k = 0
                for ha, hx in ((0, 0), (0, 1), (1, 0)):
                    for pp in range(4):
                        _mm(nc, pt,
                            at[ha][pp][:, :, d2t, :],
                            xqt[hx][qb][pp],
                            start=(k == 0), stop=(k == 11))
                        k += 1
                hs = tht[qb][d2t // 2][:, d2t % 2, :]
                ls = tlt[qb][d2t // 2][:, d2t % 2, :]
                nc.scalar.activation(hs, pt,
                                     mybir.ActivationFunctionType.Copy)
                nc.vector.tensor_sub(ls, pt, hs)

        def emit_scores(s):
            pairs = TRIPS[s] // 2
            ph_t = [pool_p.tile([P, 2, IB], F8, tag="probs",
                                name=f"ph{s}_{j}") for j in range(pairs)]
            pl_t = [pool_p.tile([P, 2, IB], F8, tag="probs",
                                name=f"pl{s}_{j}") for j in range(pairs)]
            probs[s] = (ph_t, pl_t)
            for jt in range(TRIPS[s]):
                pss = ps_a.tile([P, IB], F32, tag="ps", name=f"pss{s}_{jt}")
                k = 0
                for hx, ht in ((0, 0), (0, 1), (1, 0)):
                    tt = tht[s] if ht == 0 else tlt[s]
                    for dp in range(4):
                        _mm(nc, pss,
                            xkt[hx][:, dp, :, jt * P:(jt + 1) * P],
                            tt[dp],
                            start=(k == 0), stop=(k == 11))
                        k += 1
                kk = jt - (TRIPS[s] - 4)
                if kk >= 0:
                    nc.vector.tensor_add(pss, pss, maskt[:, s, kk, :])
                e = pool_e.tile([P, IB], F32, tag="e32", name=f"e{s}_{jt}")
                nc.scalar.activation(e, pss,
                                     mybir.ActivationFunctionType.Exp,
                                     scale=SCALE)
                hp = ph_t[jt // 2][:, jt % 2, :]
                lp = pl_t[jt // 2][:, jt % 2, :]
                nc.vector.tensor_copy(hp, e)
                nc.vector.tensor_sub(lp, e, hp)

        def emit_u(s):
            pairs = TRIPS[s] // 2
            ph_t, pl_t = probs[s]
            # softmax denominator (x16): 1-col DoubleRow matmuls
            psl = ps_l.tile([P, 2], F32, tag="psl", name=f"psl{s}")
            psls[s] = psl
            k = 0
            last = 2 * 2 * pairs - 1
            for hf in range(2):
                for j in range(pairs):
                    for t in (ph_t, pl_t):
                        _mm(nc, psl[:, hf:hf + 1],
                            t[j][:, :, hf * P:(hf + 1) * P], ones,
                            start=(k == 0), stop=(k == last))
                        k += 1
            uh_t = [pool_u.tile([P, 2, IB], F8, tag="uq",
                                name=f"uh{s}_{d}") for d in range(4)]
            ul_t = [pool_u.tile([P, 2, IB], F8, tag="uq",
                                name=f"ul{s}_{d}") for d in range(4)]
            uqs[s] = (uh_t, ul_t)
            for dt in range(8):
                psu = ps_a.tile([P, IB], F32, tag="ps", name=f"psu{s}_{dt}")
                k = 0
                for hx, hp in ((0, 0), (1, 0), (0, 1)):
                    pt_ = ph_t if hp == 0 else pl_t
                    for j in range(pairs):
                        _mm(nc, psu,
                            xnt[hx][j][:, :, dt, :], pt_[j],
                            start=(k == 0), stop=(k == 3 * pairs - 1))
                        k += 1
                hs = uh_t[dt // 2][:, dt % 2, :]
                ls = ul_t[dt // 2][:, dt % 2, :]
                nc.scalar.activation(hs, psu,
                                     mybir.ActivationFunctionType.Copy)
                nc.vector.tensor_sub(ls, psu, hs)
            rt = pool_rt.tile([P, 2], F32, tag="rt", name=f"rt{s}")
            nc.vector.reciprocal(rt, psl)
            rts[s] = rt

        def emit_out(s):
            uh_t, ul_t = uqs[s]
            for hf in range(2):
                for ob in range(4):
                    psf = ps_a.tile([P, IB], F32, tag="ps",
                                    name=f"psf{s}_{hf}_{ob}")
                    k = 0
                    for hu, hw in ((0, 0), (0, 1), (1, 0)):
                        ut = uh_t if hu == 0 else ul_t
                        for dp in range(4):
                            _mm(nc, psf,
                                ut[dp][:, :, hf * P:(hf + 1) * P],
                                wvt[hw][:, dp, :, ob * IB:(ob + 1) * IB],
                                start=(k == 0), stop=(k == 11))
                            k += 1
                    ob_t = pool_ob.tile([P, IB], BF16, tag="ob",
                                        name=f"ob{s}_{hf}_{ob}")
                    nc.vector.tensor_scalar_mul(ob_t, psf,
                                                rts[s][:, hf:hf + 1])
                    r0 = s * IB + hf * P
                    nc.sync.dma_start(
                        out=out_d[r0:r0 + P, ob * IB:(ob + 1) * IB],
                        in_=ob_t)

        emit_t(0)
        emit_t(1)
        emit_scores(0)
        emit_t(2)
        emit_scores(1)
        emit_t(3)
        emit_u(0)
        emit_scores(2)
        emit_u(1)
        emit_out(0)
        emit_scores(3)
        emit_u(2)
        emit_out(1)
        emit_u(3)
        emit_out(2)
        emit_out(3)


def _build_nc():
    nc = bacc.Bacc(None, target_bir_lowering=False)
    a_d = nc.declare_dram_parameter("a8", [2, 4, P, 2, 8, P], F8,
                                    isOutput=False)
    xq_d = nc.declare_dram_parameter("xq8", [2, 4, 4, P, 2, IB], F8,
                                     isOutput=False)
    xk_d = nc.declare_dram_parameter("xk8", [2, P, 4, 2, N], F8,
                                     isOutput=False)
    xn_d = nc.declare_dram_parameter("xn8", [2, 8, P, 2, 8, P], F8,
                                     isOutput=False)
    wv_d = nc.declare_dram_parameter("wv8", [2, P, 4, 2, NQ], F8,
                                     isOutput=False)
    mask_d = nc.declare_dram_parameter("masks", [P, 4, 4, IB], BF16,
                                       isOutput=False)
    out_d = nc.declare_dram_parameter("out_p", [NQ, D], BF16, isOutput=True)
    dram = (a_d, xq_d, xk_d, xn_d, wv_d, mask_d, out_d)

    with tile.TileContext(nc) as tc:
        with ExitStack() as ctx:
            _build_body(nc, tc, ctx, dram)
    nc.finalize()
    return nc


def _split8(a):
    h = a.astype(E4NP)
    l = (a - h.astype(np.float32)).astype(E4NP)
    return h, l


def _make_in_maps(x, W_q, W_k, W_v):
    wq = np.asarray(W_q, np.float32)
    wk = np.asarray(W_k, np.float32)
    wv = np.asarray(W_v, np.float32)
    A32 = (wq.T.astype(np.float64) @ wk.astype(np.float64) * 32.0).astype(np.float32)
    Wvt32 = np.ascontiguousarray(wv.T) * 32.0

    # a8: [2, 4 pp, 128, 2, 8, 128]
    def lhs_pack(m):       # [d1, d2] -> [4, 128, 2, 8, 128]
        return m.reshape(4, 2, P, 8, P).transpose(0, 2, 1, 3, 4)
    ah, al = _split8(A32)
    a8 = np.ascontiguousarray(np.stack([lhs_pack(ah), lhs_pack(al)]))
    # wv8: [2, 128, 4 dp, 2, 1024]
    def rhs_pack(m):       # [d, o] -> [128, 4, 2, 1024]
        return m.reshape(4, 2, P, NQ).transpose(2, 0, 1, 3)
    wh, wl = _split8(Wvt32)
    wv8 = np.ascontiguousarray(np.stack([rhs_pack(wh), rhs_pack(wl)]))

    masks = [None, None]
    for h in range(2):
        mk = _build_masks(h).astype(BFNP)       # [4, 4, 128, 256]
        masks[h] = np.ascontiguousarray(mk.transpose(2, 0, 1, 3))
    qrows = [_qrows(0), _qrows(1)]

    in_maps = []
    per_batch = {}
    for b in range(B):
        xb = np.asarray(x[b], np.float32)
        xh, xl = _split8(xb)                    # [2048, 1024] e4m3
        xh32 = xh.astype(np.float32)
        xl32 = xl.astype(np.float32)
        # xk8: [2, 128, 4 dp, 2, 2048]: from x^T [1024 d, 2048 k]
        def xk_pack(m32):
            return m32.T.reshape(4, 2, P, N).transpose(2, 0, 1, 3)
        xk8 = np.ascontiguousarray(np.stack(
            [xk_pack(xh32), xk_pack(xl32)])).astype(E4NP)
        # xn8: [2, 8 kp, 128, 2, 8 dt, 128]: from x/2 [2048 k, 1024 d]
        def xn_pack(m32):
            return (m32 * 0.5).reshape(8, 2, P, 8, P).transpose(0, 2, 1, 3, 4)
        xn8 = np.ascontiguousarray(np.stack(
            [xn_pack(xh32), xn_pack(xl32)])).astype(E4NP)
        per_batch[b] = (xh32, xl32, xk8, xn8)

    for c in range(NCORES):
        b, h = c // 2, c % 2
        xh32, xl32, xk8, xn8 = per_batch[b]
        # xq8: [2, 4 qb, 4 pp, 128, 2, 256]: x^T[:, qrows]
        def xq_pack(m32):
            # [1024 d1, 1024 q] -> [4 qb, 4 pp, 128, 2, 256]
            a = m32.T[:, qrows[h]].reshape(4, 2, P, 4, IB)
            return a.transpose(3, 0, 2, 1, 4)
        xq8 = np.ascontiguousarray(np.stack(
            [xq_pack(xh32), xq_pack(xl32)])).astype(E4NP)
        in_maps.append({
            "a8": a8, "xq8": xq8, "xk8": xk8, "xn8": xn8,
            "wv8": wv8, "masks": masks[h],
        })
    return in_maps


def kernel(x, W_q, W_k, W_v):
    global _CACHED_NC, LAST_RESULT, LAST_IN_MAPS
    x = np.asarray(x, dtype=np.float32)
    if _CACHED_NC is None:
        _CACHED_NC = _build_nc()
    nc = _CACHED_NC

    in_maps = _make_in_maps(x, W_q, W_k, W_v)
    LAST_IN_MAPS = in_maps
    try:
        res = run_bass_kernel_spmd(nc, in_maps, list(range(NCORES)))
    except Exception:
        # transient NRT_EXEC_UNIT_UNRECOVERABLE wedges clear on retry
        import time as _time
        _time.sleep(5)
        res = run_bass_kernel_spmd(nc, in_maps, list(range(NCORES)))
    LAST_RESULT = res

    qrows = [_qrows(0), _qrows(1)]
    out = np.empty((B, N, D), np.float32)
    for c in range(NCORES):
        b, h = c // 2, c % 2
        out[b, qrows[h], :] = np.asarray(res.results[c]["out_p"],
                                         dtype=np.float32)
    return out
